# revision 1
# baseline (speedup 1.0000x reference)
"""Trainium2 Bass kernel for AdaConv2d (instance-norm + per-sample dynamic
depthwise 3x3 conv + per-channel scale/bias + shared dense 3x3 conv, reflect
padding everywhere).

Data-parallel over batch: 8 samples -> 8 NeuronCores, one sample per core.
Per-sample per-channel weights shard with the batch; the shared final conv
weight/bias are replicated.

Math (per sample, per channel c):
    xn   = (x - mu_c) * rsqrt(var_c + eps)
    mid  = wp_c * depthwise3x3(reflect_pad(xn); ws_c) + b_c
         = a_c * depthwise3x3(reflect_pad(x); ws_c) + t_c
      with a_c = wp_c * rsqrt(var_c+eps),  t_c = b_c - a_c * mu_c * sum(ws_c)
    out  = dense3x3(reflect_pad(mid); conv_w) + conv_b

so the instance norm never materializes: it folds into the per-channel affine
(a, t) applied when evicting the depthwise accumulator.  The dense conv runs
as 9 shifted fp16 matmuls accumulating in PSUM (fp32); redundant LDWEIGHTS
(one per matmul in the generated stream) are deduped post-compile.

Engine split: the depthwise conv for two channel tiles runs on the tensor
engine as diagonal-weight matmuls (host-built diag matrices) — this fills
the PE pipeline while the vector engine produces the other two tiles with a
tensor_scalar (4x mode) + tensor_tensor (2x mode) ladder, so the PE never
idles long enough for the HAM clock-gate to re-throttle.
"""

import os
import sys
import types

import numpy as np

B, C, H, W = 8, 512, 64, 64
KS = 3
EPS = 1e-5
N_CORES = 8
P = 128
CT = C // P            # 4 channel tiles
PADH, PADW = H + 2, W + 2
XF = PADH * PADW       # 4356
HW = H * W             # 4096
NCHUNK = HW // 512     # 8 psum-bank chunks per output tile
PE_TILES = (2, 3)      # depthwise on TensorE (diag matmuls)
DVE_TILES = (0, 1)     # depthwise on VectorE
CI_ORDER = (2, 3, 0, 1)  # dense-conv contraction order = production order


def _install_ntff_hook():
    """Register the NTFF profiling hook that concourse expects under axon
    (missing antenv.axon_hooks module in this image)."""
    if "antenv.axon_hooks" in sys.modules:
        return
    try:
        mod = types.ModuleType("antenv.axon_hooks")
        holder = [None]
        mod.set_axon_ntff_profile_hook = lambda h: holder.__setitem__(0, h)
        mod.get_axon_ntff_profile_hook = lambda: holder[0]
        sys.modules["antenv.axon_hooks"] = mod
        from trn_agent_boot.trn_boot import _ntff_profile_via_ctypes

        hook = _ntff_profile_via_ctypes("/opt/axon/libaxon_pjrt.so")
        mod.set_axon_ntff_profile_hook(hook)
    except Exception:
        sys.modules.pop("antenv.axon_hooks", None)


_TRACE = os.environ.get("BASS_KERNEL_TRACE") == "1"
if _TRACE:
    _install_ntff_hook()

import concourse.tile as tile
from concourse import bacc, mybir
import concourse.bass_utils as bass_utils
from concourse.bass_utils import run_bass_kernel_spmd

if _TRACE:
    bass_utils.upload_artifacts = lambda d: d

LAST_EXEC_NS = None
_CACHE = {}


def _taps():
    for tap in range(KS * KS):
        yield tap, tap // KS, tap % KS


def _reflect_borders(nc, t3):
    """Fill the 1-wide reflect border of a [128, PADH, PADW] tile whose
    interior [1:H+1, 1:W+1] is already populated."""
    nc.vector.tensor_copy(t3[:, 1:H + 1, 0:1], t3[:, 1:H + 1, 2:3])
    nc.vector.tensor_copy(t3[:, 1:H + 1, PADW - 1:PADW],
                          t3[:, 1:H + 1, PADW - 3:PADW - 2])
    nc.vector.tensor_copy(t3[:, 0:1, :], t3[:, 2:3, :])
    nc.vector.tensor_copy(t3[:, PADH - 1:PADH, :], t3[:, PADH - 3:PADH - 2, :])


def _dedup_ldweights(nc):
    """Drop InstLdweights whose weights AP is identical to the previous
    weight load on the PE stream (bacc splits every matmul into LDW+MM;
    with one weight block reused across 8 PSUM chunks, 7 of 8 loads are
    redundant and serialize with the matmuls).  LDWs carrying semaphore
    waits/updates are kept."""
    n_removed = 0
    for f in nc.m.functions:
        for bb in f.blocks:
            insts = bb.instructions
            keep = []
            last_key = None
            for inst in insts:
                tn = type(inst).__name__
                if tn == "InstLdweights":
                    si = inst.sync_info
                    has_sync = si is not None and (
                        len(si.on_wait) > 0 or len(si.on_update) > 0
                    )
                    key = repr(inst.ins[0])
                    if key == last_key and not has_sync:
                        n_removed += 1
                        continue
                    last_key = key
                elif tn == "InstMatmult":
                    if getattr(inst, "is_transpose", False):
                        last_key = None
                keep.append(inst)
            if len(keep) != len(insts):
                bb.instructions = keep
    return n_removed


def _build():
    nc = bacc.Bacc("TRN2", target_bir_lowering=False, debug=False,
                   num_devices=N_CORES)
    f32 = mybir.dt.float32
    f16 = mybir.dt.float16

    x_in = nc.dram_tensor("x", [C, XF], f16, kind="ExternalInput").ap()
    prm_in = nc.dram_tensor("prm", [P, CT * 11 + CT], f32, kind="ExternalInput").ap()
    wt_in = nc.dram_tensor("wt", [CT, P, 9 * C], f16, kind="ExternalInput").ap()
    dg_in = nc.dram_tensor("dg", [P, len(PE_TILES) * 9 * P], f16,
                           kind="ExternalInput").ap()
    xsh_in = nc.dram_tensor("xsh", [len(DVE_TILES) * P, XF], f16,
                            kind="ExternalInput").ap()
    out_ext = nc.dram_tensor("out", [C, HW], f32, kind="ExternalOutput").ap()

    with tile.TileContext(nc) as tc:
        with (
            tc.tile_pool(name="wpool", bufs=1) as wpool,
            tc.tile_pool(name="xpool", bufs=4) as xpool,
            tc.tile_pool(name="midpool", bufs=1) as midpool,
            tc.tile_pool(name="accpool", bufs=3) as accpool,
            tc.tile_pool(name="ypool", bufs=4) as ypool,
            tc.tile_pool(name="smpool", bufs=8) as smpool,
            tc.tile_pool(name="prmpool", bufs=4) as prmpool,
            tc.tile_pool(name="opool", bufs=3) as opool,
            tc.tile_pool(name="psum", bufs=4, space="PSUM") as psum,
        ):
            # x arrives host-side reflect-padded ([C, 66*66], contiguous
            # per channel) so each tile loads as one descriptor/partition.
            # diag weights first (PE work starts the moment they land).
            x_pads = {}

            # each engine owns its own DMA queue (~90 GB/s each); issuing
            # large transfers round-robin across engines runs them in
            # parallel instead of serializing on the sync queue.
            x_queues = {2: nc.sync, 3: nc.sync, 0: nc.scalar, 1: nc.scalar}

            def load_x(t, split=False, defer=False):
                xp = xpool.tile([P, XF], f16, name="xpad", tag="xpad")
                x3 = xp.rearrange("p (h w) -> p h w", h=PADH)
                x_pads[t] = (xp, x3)
                if defer:
                    return xp, x3
                if split:
                    # halves let the depthwise matmuls start on the first
                    # rows while the rest is still in flight (subtile deps)
                    hhalf = (PADH // 2) * PADW
                    x_queues[t].dma_start(xp[:, 0:hhalf],
                                          x_in[t * P:(t + 1) * P, 0:hhalf])
                    x_queues[t].dma_start(xp[:, hhalf:XF],
                                          x_in[t * P:(t + 1) * P, hhalf:XF])
                else:
                    x_queues[t].dma_start(xp[:], x_in[t * P:(t + 1) * P, :])
                return xp, x3

            dg_sb = wpool.tile([P, len(PE_TILES) * 9 * P], f16,
                               name="dg_sb", tag="dg")
            nc.scalar.dma_start(dg_sb[:], dg_in[:])
            prm_all = prmpool.tile([P, CT * 11 + CT], f32, name="prm_all",
                                   tag="prm")
            nc.scalar.dma_start(prm_all[:], prm_in[:])
            load_x(PE_TILES[0], split=True)
            load_x(PE_TILES[1], split=True)
            load_x(DVE_TILES[0])
            # x for the second DVE tile is deliberately loaded late (after
            # the dense weights): if it lands early, the scalar engine
            # greedily runs its stats passes ahead of the PE-tile psum
            # evictions and stalls the tensor engine on psum banks.
            load_x(DVE_TILES[1], defer=True)
            xshs = {}
            for j, t in enumerate(DVE_TILES):
                xsh = xpool.tile([P, XF], f16, name="xsh", tag="xsh", bufs=2)
                nc.scalar.dma_start(xsh[:], xsh_in[j * P:(j + 1) * P, :])
                xshs[t] = xsh.rearrange("p (h w) -> p h w", h=PADH)

            # dense-conv weights on the sync queue behind the x halves, and
            # the second DVE tile's x behind those: it must not land before
            # the PE-tile psum evictions are ready, or the scalar engine
            # greedily runs its stats passes first and stalls the PE.
            w_sb = []
            for t in range(CT):
                w = wpool.tile([P, 9 * C], f16, name=f"w{t}", tag=f"w{t}")
                nc.sync.dma_start(w[:], wt_in[t])
                w_sb.append(w.rearrange("p (k c) -> p k c", k=9))
            t1_late = DVE_TILES[1]
            nc.sync.dma_start(x_pads[t1_late][0][:],
                              x_in[t1_late * P:(t1_late + 1) * P, :])

            prms = [prm_all[:, t * 11:(t + 1) * 11] for t in range(CT)]
            cb_sb = prm_all[:, CT * 11:CT * 11 + CT]

            # HAM warm-up: ~5us of throwaway matmuls on a memset tile so the
            # PE clock-gate reaches 8/8 before the real depthwise work.
            # (no DMA dependency -> starts right after the preamble)
            # (no explicit HAM warm-up: the depthwise matmuls start early
            # enough to warm the clock-gate themselves)

            mid = []
            for t in range(CT):
                m = midpool.tile([P, XF], f16, name=f"mid{t}", tag=f"mid{t}")
                mid.append(m.rearrange("p (h w) -> p h w", h=PADH))

            def stats(t, x3, scratch3, on_dve=False):
                """mean/var of the tile -> per-channel affine (a, tb).
                scratch3 is a [P,H,W]-viewed fp16 scratch that receives the
                squares (overwritten later).  on_dve runs the two big
                reduction passes on the vector engine (scalar_tensor_tensor
                with accum_out) so the PE-tile evictions don't queue behind
                the scalar engine's serial stats chain."""
                prm = prms[t]
                sqs = smpool.tile([P, 1], f32, name="sqs", tag="sm")
                ms = smpool.tile([P, 1], f32, name="ms", tag="sm")
                xin = x3[:, 1:H + 1, 1:W + 1]
                sscr = ypool.tile([P, HW], f16, name="y", tag="y")
                sscr3 = sscr.rearrange("p (h w) -> p h w", h=H)
                if on_dve:
                    # sum(x^2): out = (x * 1) * x ; accum = rowsum
                    nc.vector.scalar_tensor_tensor(
                        scratch3, xin, 1.0, xin,
                        mybir.AluOpType.mult, mybir.AluOpType.mult,
                        accum_out=sqs[:],
                    )
                    # sum(x): out = (x * 0) + x ; accum = rowsum
                    nc.vector.scalar_tensor_tensor(
                        sscr3, xin, 0.0, xin,
                        mybir.AluOpType.mult, mybir.AluOpType.add,
                        accum_out=ms[:],
                    )
                else:
                    nc.scalar.activation(
                        scratch3, xin,
                        mybir.ActivationFunctionType.Square, accum_out=sqs[:],
                    )
                    nc.scalar.activation(
                        sscr3, xin,
                        mybir.ActivationFunctionType.Identity, accum_out=ms[:],
                    )
                mu = smpool.tile([P, 1], f32, name="mu", tag="sm")
                nc.vector.tensor_scalar_mul(mu[:], ms[:], 1.0 / HW)
                ex2 = smpool.tile([P, 1], f32, name="ex2", tag="sm")
                nc.vector.tensor_scalar_mul(ex2[:], sqs[:], 1.0 / HW)
                mu2 = smpool.tile([P, 1], f32, name="mu2", tag="sm")
                nc.vector.tensor_mul(mu2[:], mu[:], mu[:])
                ve = smpool.tile([P, 1], f32, name="ve", tag="sm")
                nc.vector.scalar_tensor_tensor(
                    ve[:], mu2[:], -1.0, ex2[:],
                    mybir.AluOpType.mult, mybir.AluOpType.add,
                )
                nc.vector.tensor_scalar_add(ve[:], ve[:], EPS)
                sd = smpool.tile([P, 1], f32, name="sd", tag="sm")
                nc.scalar.sqrt(sd[:], ve[:])
                r = smpool.tile([P, 1], f32, name="r", tag="sm")
                nc.vector.reciprocal(r[:], sd[:])
                a = smpool.tile([P, 1], f32, name="a", tag="a")
                nc.vector.tensor_mul(a[:], r[:], prm[:, 9:10])
                s9 = smpool.tile([P, 1], f32, name="s9", tag="sm")
                nc.vector.tensor_reduce(
                    s9[:], prm[:, 0:9], mybir.AxisListType.X,
                    mybir.AluOpType.add,
                )
                am = smpool.tile([P, 1], f32, name="am", tag="sm")
                nc.vector.tensor_mul(am[:], a[:], mu[:])
                tb = smpool.tile([P, 1], f32, name="tb", tag="tb")
                nc.vector.scalar_tensor_tensor(
                    tb[:], am[:], 1.0, s9[:],
                    mybir.AluOpType.mult, mybir.AluOpType.mult,
                )
                nc.vector.scalar_tensor_tensor(
                    tb[:], tb[:], -1.0, prm[:, 10:11],
                    mybir.AluOpType.mult, mybir.AluOpType.add,
                )
                return a, tb

            # ---- depthwise on PE via diagonal-weight matmuls -------------
            for j, t in enumerate(PE_TILES):
                xp, x3 = x_pads[t]
                scr = accpool.tile([P, HW], f16, name="acc", tag="acc")
                # high_priority keeps the whole stats chain at the head of
                # the vector-engine stream: if ladder ops interleave, the
                # evictions' semaphore thresholds only clear after those
                # unrelated ops and the tensor engine stalls on psum banks
                with tc.high_priority():
                    a, tb = stats(t, x3,
                                  scr.rearrange("p (h w) -> p h w", h=H),
                                  on_dve=True)
                # two half-passes: the first half's psum evictions run
                # concurrently with the second half's matmuls, shortening
                # the eviction chain the next bank consumer waits on
                for hf in range(2):
                    banks = [
                        psum.tile([P, 1024], f32, name="bank", tag="bank")
                        for _ in range(2)
                    ]
                    for tap, dy, dx in _taps():
                        dgv = dg_sb[:, (j * 9 + tap) * P:(j * 9 + tap + 1) * P]
                        for lc in range(4):
                            ch = hf * 4 + lc
                            rhs = x3[:, ch * 8 + dy:ch * 8 + dy + 8, dx:dx + W]
                            half = (lc % 2) * 512
                            nc.tensor.matmul(
                                banks[lc // 2][:, half:half + 512], dgv, rhs,
                                start=(tap == 0), stop=(tap == 8),
                            )
                    for cp in range(2):
                        r0 = (hf * 4 + 2 * cp) * 8
                        nc.scalar.activation(
                            mid[t][:, 1 + r0:1 + r0 + 16, 1:W + 1],
                            banks[cp][:],
                            mybir.ActivationFunctionType.Identity,
                            bias=tb[:], scale=a[:],
                        )
                _reflect_borders(nc, mid[t])

            # ---- depthwise on DVE: tensor_scalar (4x) + tensor_tensor (2x)
            for t in DVE_TILES:
                xp, x3 = x_pads[t]
                acc = accpool.tile([P, HW], f16, name="acc", tag="acc")
                av = acc.rearrange("p (h w) -> p h w", h=H)
                # square scratch must NOT be acc: the depthwise ladder would
                # serialize behind the (busy) scalar engine's stats chain.
                sqscr = ypool.tile([P, HW], f16, name="y", tag="y")
                # pin the second tile's stats past the PE-tile evictions in
                # the scheduler's simulated timeline: its static ACT order
                # otherwise places them first (the cost model underestimates
                # how late this tile's x lands) and the eviction chain then
                # stalls the tensor engine on psum-bank reuse.
                with tc.tile_wait_until(0.030, enable=(t == DVE_TILES[1])):
                    a, tb = stats(t, x3,
                                  sqscr.rearrange("p (h w) -> p h w", h=H))
                prm = prms[t]
                # tap0 (dx=0, aligned): tensor_scalar 4x mode; other aligned
                # taps via tensor_scalar+tensor_add (4x + 2x); dx==1 taps are
                # 2-byte-misaligned so use scalar_tensor_tensor (1x mode).
                nc.vector.tensor_scalar_mul(av[:], x3[:, 0:H, 0:W],
                                            prm[:, 0:1])
                act_taps = (2, 5, 8) if t == DVE_TILES[1] else ()
                for tap, dy, dx in _taps():
                    if tap == 0:
                        continue
                    # dx==1 reads are 2-byte-misaligned in x3; the host-built
                    # column-shifted copy (xsh) keeps them packed-mode.
                    xv = (xshs[t][:, dy:dy + H, 0:W] if dx == 1
                          else x3[:, dy:dy + H, dx:dx + W])
                    y = ypool.tile([P, HW], f16, name="y", tag="y")
                    yv = y.rearrange("p (h w) -> p h w", h=H)
                    if tap in act_taps:
                        # the scalar engine is idle in this window; scaled
                        # copies there shorten the vector-engine chain
                        nc.scalar.activation(
                            yv[:], xv, mybir.ActivationFunctionType.Copy,
                            scale=prm[:, tap:tap + 1],
                        )
                    else:
                        nc.vector.tensor_scalar_mul(yv[:], xv,
                                                    prm[:, tap:tap + 1])
                    nc.vector.tensor_add(acc[:], acc[:], y[:])
                nc.scalar.activation(
                    mid[t][:, 1:H + 1, 1:W + 1], av[:],
                    mybir.ActivationFunctionType.Identity,
                    bias=tb[:], scale=a[:],
                )
                _reflect_borders(nc, mid[t])


            # ---- dense 3x3: out[co] = sum_{ci,tap} w^T @ shifted(mid[ci])
            for co in range(CT):
                banks = [
                    psum.tile([P, 1024], f32, name="bank", tag="bank")
                    for _ in range(NCHUNK // 2)
                ]
                for ci_i, ci in enumerate(CI_ORDER):
                    for tap, dy, dx in _taps():
                        w_view = w_sb[ci][:, tap, co * P:(co + 1) * P]
                        for ch in range(NCHUNK):
                            rhs = mid[ci][:, ch * 8 + dy:ch * 8 + dy + 8,
                                          dx:dx + W]
                            half = (ch % 2) * 512
                            nc.tensor.matmul(
                                banks[ch // 2][:, half:half + 512], w_view, rhs,
                                start=(ci_i == 0 and tap == 0),
                                stop=(ci_i == CT - 1 and tap == 8),
                            )
                for cp in range(NCHUNK // 2):
                    o = opool.tile([P, 1024], f32, name="o", tag="o")
                    nc.scalar.activation(
                        o[:], banks[cp][:],
                        mybir.ActivationFunctionType.Identity,
                        bias=cb_sb[:, co:co + 1], scale=1.0,
                    )
                    nc.sync.dma_start(
                        out_ext[co * P:(co + 1) * P,
                                cp * 1024:(cp + 1) * 1024],
                        o[:],
                    )

    nc.compile()
    _dedup_ldweights(nc)
    return nc


def kernel(x, w_spatial, w_pointwise, bias, conv_w, conv_b):
    global LAST_EXEC_NS
    if "nc" not in _CACHE:
        _CACHE["nc"] = _build()
    nc = _CACHE["nc"]

    xf = np.asarray(x, dtype=np.float32).astype(np.float16)
    x16 = np.ascontiguousarray(
        np.pad(xf, ((0, 0), (0, 0), (1, 1), (1, 1)), mode="reflect"))
    ws = np.asarray(w_spatial, dtype=np.float32).reshape(B, C, 9)
    wp = np.asarray(w_pointwise, dtype=np.float32).reshape(B, C)
    bi = np.asarray(bias, dtype=np.float32).reshape(B, C)
    cw = np.asarray(conv_w, dtype=np.float32)
    cb = np.asarray(conv_b, dtype=np.float32)

    # shared final-conv weight, laid out for lhsT views:
    # wt[t, p, tap, co] = conv_w[co, t*128+p, tap//3, tap%3]
    wt = np.ascontiguousarray(
        cw.reshape(C, CT, P, 9).transpose(1, 2, 3, 0).astype(np.float16)
    ).reshape(CT, P, 9 * C)
    cbl = np.ascontiguousarray(cb.reshape(CT, P).T)  # [P, CT]

    idx = np.arange(P)
    in_maps = []
    for b in range(B):
        prm = np.empty((CT, P, 11), dtype=np.float32)
        prm[:, :, 0:9] = ws[b].reshape(CT, P, 9)
        prm[:, :, 9] = wp[b].reshape(CT, P)
        prm[:, :, 10] = bi[b].reshape(CT, P)
        xpb = x16[b, DVE_TILES[0] * P:(DVE_TILES[-1] + 1) * P]
        xsh = np.ascontiguousarray(np.concatenate(
            [xpb[:, :, 1:], xpb[:, :, -1:]], axis=2)).reshape(-1, XF)
        prm = np.concatenate(
            [prm.transpose(1, 0, 2).reshape(P, CT * 11), cbl], axis=1)
        prm = np.ascontiguousarray(prm)
        # diagonal depthwise weight matrices for the PE tiles
        dg = np.zeros((len(PE_TILES) * 9, P, P), dtype=np.float16)
        for j, t in enumerate(PE_TILES):
            for tap in range(9):
                dg[j * 9 + tap, idx, idx] = ws[b, t * P:(t + 1) * P, tap]
        dg = np.ascontiguousarray(
            dg.transpose(1, 0, 2).reshape(P, len(PE_TILES) * 9 * P))
        in_maps.append({
            "x": x16[b].reshape(C, XF),
            "prm": prm,
            "wt": wt,
            "dg": dg,
            "xsh": xsh,
        })

    res = run_bass_kernel_spmd(
        nc, in_maps, list(range(N_CORES)), trace=_TRACE
    )
    LAST_EXEC_NS = res.exec_time_ns
    out = np.stack([res.results[b]["out"].reshape(C, H, W) for b in range(B)])
    return out

